# revision 1
# baseline (speedup 1.0000x reference)
"""GQA attention block (B=2, L=2048, D=4096, H=32, HKV=8, RoPE, causal) on 8
Trainium2 NeuronCores.

Sharding: core c -> batch b=c//4, head-group g=c%4 (8 Q heads + 2 KV heads per
core).  Each core computes x[b] @ wq_g/wk_g/wv_g projections, RoPE, causal
attention for its heads, and a partial output projection against its slice of
wo (row-sharded contraction).  The host sums the 4 partials per batch element
(the all-reduce of the tensor-parallel output projection, done at unshard).

Device layouts put every matmul contraction on the partition axis; the host
pre-tiles x and all weights into the exact SBUF tile layouts so every DMA is
a single fully-contiguous read.  wq/wk rows are pair-permuted ([evens|odds]
per head) so RoPE becomes a partition half-swap, folded into partition-offset
operands of the sin multiply (no explicit swap copies).

Scores are computed transposed, S^T[j, l] = K^T.T @ Q^T, so softmax probs
feed the PV matmul with no on-chip transposes.  The softmax denominator
accumulates through an all-ones stationary matmul over the same E^T tiles
(partition-broadcast for free); normalization is reciprocal_approx_fast +
multiply.  Causality: fully-masked key tiles are skipped; diagonal tiles are
zeroed post-exp with a gpsimd affine_select (exp(s+m) == exp(s)*[m==0]
exactly for the 0/-1e9 mask).  Score matmuls are issued LOOKAHEAD tiles
ahead of the PV/denominator matmuls so the scalar-engine exp latency stays
off the PE critical path.

All matmuls run in float32r (full-rate fp32 mode at moving-dim 512,
~1.6e-4 rel err measured on hardware).
"""

import numpy as np

import concourse.mybir as mybir
import concourse.tile as tile
from concourse import bacc, bass_utils

B, L, D = 2, 2048, 4096
H, HKV, HD = 32, 8, 128
NCORES = 8
GROUPS = 4                # head groups (cores per batch element)
QH = H // GROUPS          # 8 q heads per core
KVH = HKV // GROUPS       # 2 kv heads per core
NM = QH + 2 * KVH         # 12 projection m-tiles per core (q0..7, k0..1, v0..1)
LC = 512                  # l-chunk (matmul moving free dim)
DT = D // 128             # 32 contraction tiles for projections
SCALE = 1.0 / float(np.sqrt(HD))
LOOKAHEAD = 3             # score-matmul tiles in flight ahead of PV

f32 = mybir.dt.float32
f32r = mybir.dt.float32r


def build_nc(seq_len=L):
    nlc = seq_len // LC
    njt_all = seq_len // 128

    lc_pairs0 = [
        [lc for lc in (2 * i, 2 * i + 1) if lc < nlc] for i in range((nlc + 1) // 2)
    ]
    max_plc = max(len(p) for p in lc_pairs0)
    nc = bacc.Bacc(trn_type="TRN2")
    # host-pre-tiled operands: every DMA below is a contiguous read
    x_tl = nc.dram_tensor(
        "x_tl", [len(lc_pairs0) * 8, 128, 4 * max_plc * LC], f32, kind="ExternalInput"
    )
    wqkv_tl = nc.dram_tensor(
        "wqkv_tl", [NM * 2, 128, 16 * 128], f32, kind="ExternalInput"
    )
    wo_tl = nc.dram_tensor("wo_tl", [D // 128, 128, QH * 128], f32, kind="ExternalInput")
    cosT = nc.dram_tensor("cosT", [64, seq_len], f32, kind="ExternalInput")
    sinT = nc.dram_tensor("sinT", [64, seq_len], f32, kind="ExternalInput")
    ones128 = nc.dram_tensor("ones128", [128, 128], f32, kind="ExternalInput")
    ident = nc.dram_tensor("ident", [128, 128], f32, kind="ExternalInput")
    outT = nc.dram_tensor("outT", [D, seq_len], f32, kind="ExternalOutput")

    with tile.TileContext(nc) as tc:
        with (
            tc.tile_pool(name="persist", bufs=1) as pp,
            tc.tile_pool(name="xp", bufs=1) as xp,
            tc.tile_pool(name="qp", bufs=1) as qp,
            tc.tile_pool(name="op", bufs=1) as op_,
            tc.tile_pool(name="wp", bufs=2) as wp,
            tc.tile_pool(name="ep", bufs=2) as ep,
            tc.tile_pool(name="tp", bufs=1) as tp,
            tc.tile_pool(name="outp", bufs=1) as outp,
            tc.tile_pool(name="mmps", bufs=4, space="PSUM") as mmps,
            tc.tile_pool(name="ops", bufs=2, space="PSUM") as ops_,
            tc.tile_pool(name="dps", bufs=2, space="PSUM") as dps,
        ):
            kT_t = {
                (kv, lc): pp.tile(
                    [128, LC], f32r, tag=f"kT_{kv}_{lc}", name=f"kT_{kv}_{lc}"
                )
                for kv in range(KVH) for lc in range(nlc)
            }
            v_t = {
                lc: pp.tile(
                    [128, 4, KVH * HD], f32r, tag=f"v_{lc}", name=f"v_{lc}"
                )
                for lc in range(nlc)
            }
            cs2 = pp.tile([128, seq_len], f32)
            sn2 = pp.tile([128, seq_len], f32)
            o128 = pp.tile([128, 128], f32r)
            idt = pp.tile([128, 128], f32r)

            nc.scalar.dma_start(cs2[0:64, :], cosT.ap())
            nc.scalar.dma_start(cs2[64:128, :], cosT.ap())
            nc.scalar.dma_start(sn2[0:64, :], sinT.ap())
            nc.scalar.dma_start(sn2[64:128, :], sinT.ap())
            # rotate-half form: out = q*cs2 + swap(q)*sn2 with sn2 = [-sin | sin]
            nc.vector.tensor_scalar_mul(sn2[0:64, :], sn2[0:64, :], -1.0)
            nc.scalar.dma_start(o128[:], ones128.ap().bitcast(f32r))
            nc.scalar.dma_start(idt[:], ident.ap().bitcast(f32r))

            lc_pairs = [
                [lc for lc in (2 * i, 2 * i + 1) if lc < nlc]
                for i in range((nlc + 1) // 2)
            ]

            for pi, lcs in enumerate(lc_pairs):
                plc = len(lcs)
                o2 = op_.tile([128, QH, plc * LC], f32r, tag="o2")
                q_pr = qp.tile([128, QH, plc * LC], f32r, tag="q")
                # ---- phase 1: projections for the pair, contraction dt-halves ----
                for half in range(2):
                    x_h = xp.tile([128, DT // 2, plc * LC], f32r, tag="x")
                    for quar in range(4):
                        nc.sync.dma_start(
                            x_h[:, quar * 4:(quar + 1) * 4, :],
                            x_tl.ap()[pi * 8 + half * 4 + quar]
                            .rearrange("p (a b) -> p a b", a=4)[:, :, : plc * LC]
                            .bitcast(f32r),
                        )
                    for mi in range(NM):
                        kind = "q" if mi < QH else ("k" if mi < QH + KVH else "v")
                        m = mi if mi < QH else (mi - QH if kind == "k" else mi - QH - KVH)
                        wt = wp.tile([128, 16 * 128], f32r, tag="w")
                        nc.sync.dma_start(
                            wt[:], wqkv_tl.ap()[mi * 2 + half].bitcast(f32r)
                        )
                        for lci, lc in enumerate(lcs):
                            ps = mmps.tile([128, LC], f32, tag="mm")
                            for dt8 in range(16):
                                nc.tensor.matmul(
                                    ps[:],
                                    wt[:, dt8 * 128:(dt8 + 1) * 128],
                                    x_h[:, dt8, lci * LC:(lci + 1) * LC],
                                    start=(dt8 == 0), stop=(dt8 == 15),
                                )
                            if kind in ("q", "k"):
                                lsl = slice(lc * LC, (lc + 1) * LC)
                                t1 = tp.tile([128, LC], f32, tag="t1")
                                nc.vector.tensor_mul(t1[:], ps[:], cs2[:, lsl])
                                t2 = tp.tile([128, LC], f32, tag="t2")
                                nc.vector.tensor_mul(
                                    t2[0:64, :], ps[64:128, :], sn2[0:64, lsl]
                                )
                                nc.vector.tensor_mul(
                                    t2[64:128, :], ps[0:64, :], sn2[64:128, lsl]
                                )
                                dst = (
                                    q_pr[:, m, lci * LC:(lci + 1) * LC]
                                    if kind == "q"
                                    else kT_t[(m, lc)][:]
                                )
                                if half == 0:
                                    nc.vector.tensor_tensor(
                                        dst, t1[:], t2[:], mybir.AluOpType.add
                                    )
                                else:
                                    nc.vector.tensor_tensor(
                                        dst, dst, t1[:], mybir.AluOpType.add
                                    )
                                    nc.vector.tensor_tensor(
                                        dst, dst, t2[:], mybir.AluOpType.add
                                    )
                            else:
                                vt = tp.tile([128, LC], f32r, tag="vt")
                                nc.vector.tensor_copy(vt[:], ps[:])
                                for jj in range(4):
                                    pt = mmps.tile([128, 128], f32r, tag="mm")
                                    nc.tensor.transpose(
                                        pt[:], vt[:, jj * 128:(jj + 1) * 128], idt[:]
                                    )
                                    dstv = v_t[lc][:, jj, m * 128:(m + 1) * 128]
                                    if half == 0:
                                        nc.vector.tensor_copy(dstv, pt[:])
                                    else:
                                        nc.vector.tensor_tensor(
                                            dstv, dstv, pt[:], mybir.AluOpType.add
                                        )
                for lci, lc in enumerate(lcs):
                    # ---- phase 2: causal attention for queries in this l-chunk ----
                    njt = 4 * (lc + 1)
                    for h in range(QH):
                        kv = h // (QH // KVH)
                        po = ops_.tile([128, LC], f32, tag="po")
                        pden = dps.tile([128, LC], f32, tag="pden")
                        e_tiles = {}

                        def emit_score(jt, h=h, kv=kv, e_tiles=e_tiles, lc=lc):
                            psS = mmps.tile([128, LC], f32, tag="mm")
                            nc.tensor.matmul(
                                psS[:],
                                kT_t[(kv, jt // 4)][:, (jt % 4) * 128:(jt % 4 + 1) * 128],
                                q_pr[:, h, lci * LC:(lci + 1) * LC],
                                start=True, stop=True,
                            )
                            e = ep.tile([128, LC], f32r, tag="e")
                            nc.scalar.activation(
                                e[:], psS[:], mybir.ActivationFunctionType.Exp,
                                scale=SCALE,
                            )
                            dg = jt - 4 * lc
                            if dg >= 0:
                                # causal: zero E where key j > query l
                                # value(p, y) = -p + y - 128*dg ; keep when >= 0
                                nc.gpsimd.affine_select(
                                    out=e[:], in_=e[:],
                                    compare_op=mybir.AluOpType.is_ge,
                                    fill=0.0,
                                    base=-128 * dg,
                                    pattern=[[1, LC]],
                                    channel_multiplier=-1,
                                )
                            e_tiles[jt] = e

                        for jt in range(min(LOOKAHEAD, njt)):
                            emit_score(jt)
                        for jt in range(njt):
                            if jt + LOOKAHEAD < njt:
                                emit_score(jt + LOOKAHEAD)
                            e = e_tiles.pop(jt)
                            nc.tensor.matmul(
                                po[:],
                                v_t[jt // 4][:, jt % 4, kv * 128:(kv + 1) * 128],
                                e[:],
                                start=(jt == 0), stop=(jt == njt - 1),
                            )
                            nc.tensor.matmul(
                                pden[:], o128[:], e[:],
                                start=(jt == 0), stop=(jt == njt - 1),
                            )
                        rec = tp.tile([128, LC], f32, tag="rec")
                        nc.vector.reciprocal_approx_fast(out=rec[:], in_=pden[:])
                        nc.vector.tensor_mul(
                            o2[:, h, lci * LC:(lci + 1) * LC], po[:], rec[:]
                        )
                # ---- phase 3: partial output projection for the pair ----
                for nt in range(D // 128):
                    wo_t = wp.tile([128, QH * 128], f32r, tag="w")
                    nc.scalar.dma_start(wo_t[:], wo_tl.ap()[nt].bitcast(f32r))
                    for lci, lc in enumerate(lcs):
                        pso = mmps.tile([128, LC], f32, tag="mm")
                        for h in range(QH):
                            nc.tensor.matmul(
                                pso[:], wo_t[:, h * 128:(h + 1) * 128],
                                o2[:, h, lci * LC:(lci + 1) * LC],
                                start=(h == 0), stop=(h == QH - 1),
                            )
                        ob = outp.tile([128, LC], f32, tag="ob")
                        nc.vector.tensor_copy(ob[:], pso[:])
                        nc.sync.dma_start(
                            outT.ap()[nt * 128:(nt + 1) * 128, lc * LC:(lc + 1) * LC],
                            ob[:],
                        )
    nc.compile()
    return nc


_PERM = np.concatenate([np.arange(0, HD, 2), np.arange(1, HD, 2)])


def _tile_weight(wT):
    """[D, M] (transposed weight) -> [M//128 * 2, 128, 16*128] contiguous tiles:
    tile (m, half)[p, dt8, mc] = wT[(half*16+dt8)*128 + p, m*128 + mc]."""
    Dd, M = wT.shape
    w = wT.reshape(2, 16, 128, M // 128, 128)         # [half, dt8, p, m, mc]
    w = w.transpose(3, 0, 2, 1, 4)                     # [m, half, p, dt8, mc]
    return np.ascontiguousarray(w.reshape(M // 128 * 2, 128, 16 * 128), np.float32)


def shard_inputs(x, wq, wk, wv, wo, cos, sin, mask, seq_len=L):
    """Build the 8 per-core input maps (host pre-tiling)."""
    nlc = seq_len // LC
    cosT = np.ascontiguousarray(cos[:seq_len].T, dtype=np.float32)
    sinT = np.ascontiguousarray(sin[:seq_len].T, dtype=np.float32)
    ones128 = np.ones((128, 128), np.float32)
    ident = np.eye(128, dtype=np.float32)

    lc_pairs = [
        [lc for lc in (2 * i, 2 * i + 1) if lc < nlc] for i in range((nlc + 1) // 2)
    ]
    max_plc = max(len(p) for p in lc_pairs)
    x_tls = []
    for b in range(B):
        xT = x[b, :seq_len].T.astype(np.float32)       # [D, seq]
        xv = xT.reshape(8, 4, 128, seq_len)            # [hq(half*4+quar), dt4, p, l]
        x_tl = np.zeros((len(lc_pairs) * 8, 128, 4 * max_plc * LC), np.float32)
        for pi, lcs in enumerate(lc_pairs):
            cols = np.concatenate([np.arange(lc * LC, (lc + 1) * LC) for lc in lcs])
            blk = xv[:, :, :, cols]                    # [hq, dt4, p, plc*LC]
            blk = blk.transpose(0, 2, 1, 3)            # [hq, p, dt4, plc*LC]
            x_tl[pi * 8:(pi + 1) * 8, :, : len(cols) * 4] = blk.reshape(8, 128, -1)
        x_tls.append(x_tl)

    def permute_rows(w):
        nh = w.shape[0] // HD
        wp_ = w.reshape(nh, HD, -1)[:, _PERM, :]
        return wp_.reshape(w.shape)

    in_maps = []
    for c in range(NCORES):
        b, g = divmod(c, GROUPS)
        wq_g = permute_rows(wq[QH * HD * g:QH * HD * (g + 1)])
        wk_g = permute_rows(wk[KVH * HD * g:KVH * HD * (g + 1)])
        wv_g = wv[KVH * HD * g:KVH * HD * (g + 1)]
        wo_g = wo[:, QH * HD * g:QH * HD * (g + 1)]
        wqkv_tl = np.concatenate(
            [_tile_weight(wq_g.T), _tile_weight(wk_g.T), _tile_weight(wv_g.T)], axis=0
        )
        woT = wo_g.T.astype(np.float32)                # [1024, D]
        wov = woT.reshape(QH, 128, D // 128, 128)      # [kt, p, nt, n]
        wov = wov.transpose(2, 1, 0, 3)                # [nt, p, kt, n]
        wo_tl = np.ascontiguousarray(wov.reshape(D // 128, 128, QH * 128), np.float32)
        in_maps.append({
            "x_tl": x_tls[b],
            "wqkv_tl": wqkv_tl,
            "wo_tl": wo_tl,
            "cosT": cosT,
            "sinT": sinT,
            "ones128": ones128,
            "ident": ident,
        })
    return in_maps


def gather_output(results, seq_len=L):
    out = np.zeros((B, seq_len, D), np.float32)
    for c in range(NCORES):
        b = c // GROUPS
        out[b] += results[c]["outT"].T
    return out


_nc_cache = {}


def _get_nc(seq_len=L):
    if seq_len not in _nc_cache:
        _nc_cache[seq_len] = build_nc(seq_len)
    return _nc_cache[seq_len]


def run_sharded(inputs, trace=False, tmpdir=None):
    nc = _get_nc()
    in_maps = shard_inputs(**inputs)
    res = bass_utils.run_bass_kernel_spmd(
        nc, in_maps, core_ids=list(range(NCORES)), trace=trace, tmpdir=tmpdir
    )
    return gather_output(res.results), res


def kernel(**inputs) -> np.ndarray:
    out, _ = run_sharded(inputs)
    return out



# revision 9
# speedup vs baseline: 1.5708x; 1.5708x over previous
"""GQA attention block (B=2, L=2048, D=4096, H=32, HKV=8, RoPE, causal) on 8
Trainium2 NeuronCores.

Sharding: core c -> batch b=c//4, head-group g=c%4 (8 Q heads + 2 KV heads per
core).  Each core computes x[b] @ wq_g/wk_g projections, V^T directly (by
swapping matmul operands: stationary x-tile, moving wv^T), RoPE, causal
attention for its heads, and a partial output projection against its slice of
wo; the host sums the 4 partials per batch element.

All DMA'd operands are bf16 (half the HBM traffic of fp32; matmul rate on TRN2
is 1 row/cycle for bf16 and fp32r alike).  Projections and the output
projection use 1024-wide moving dims (both 512-l-chunks of a pair per
instruction).  Scores are computed transposed S^T[j,l] in pairs of key tiles
per PSUM allocation ([128,1024] spanning 2 banks) so one exp activation covers
two tiles; softmax denominator accumulates through an all-ones bf16 stationary
matmul over the same e tiles.  Causality: fully-masked key tiles are skipped;
diagonal tiles are zeroed post-exp with gpsimd affine_select.  The attention
inner loop is software-pipelined flat across (head, key-group) items with a
lookahead so the scalar-engine exp stays off the PE critical path.  Output
projection DMAs go directly PSUM -> DRAM.
"""

from collections import deque

import numpy as np
import ml_dtypes

import concourse.mybir as mybir
import concourse.tile as tile
from concourse import bacc, bass_utils

B, L, D = 2, 2048, 4096
H, HKV, HD = 32, 8, 128
NCORES = 8
GROUPS = 4                # head groups (cores per batch element)
QH = H // GROUPS          # 8 q heads per core
KVH = HKV // GROUPS       # 2 kv heads per core
LC = 512                  # l-chunk
SCALE = 1.0 / float(np.sqrt(HD))
LOOKAHEAD = 2             # attention items (head, key-pair) in flight ahead of PV

f32 = mybir.dt.float32
bf16 = mybir.dt.bfloat16
bfdt = ml_dtypes.bfloat16


def build_nc(seq_len=L):
    nlc = seq_len // LC            # 4
    npair = nlc // 2               # 2

    nc = bacc.Bacc(trn_type="TRN2")
    x_tl = nc.dram_tensor("x_tl", [npair * 4, 128, 8, 1024], bf16, kind="ExternalInput")
    wqk_tl = nc.dram_tensor("wqk_tl", [KVH + QH, 128, 32 * 128], bf16, kind="ExternalInput")
    wvT_tl = nc.dram_tensor("wvT_tl", [128, 32, KVH * HD], bf16, kind="ExternalInput")
    wo_tl = nc.dram_tensor("wo_tl", [D // 128, 128, QH * 128], bf16, kind="ExternalInput")
    cosT = nc.dram_tensor("cosT", [64, seq_len], f32, kind="ExternalInput")
    sinT = nc.dram_tensor("sinT", [64, seq_len], f32, kind="ExternalInput")
    ones128 = nc.dram_tensor("ones128", [128, 128], bf16, kind="ExternalInput")
    outT = nc.dram_tensor("outT", [D, seq_len], f32, kind="ExternalOutput")

    with nc.allow_low_precision("bf16 attention kernel"), tile.TileContext(nc) as tc:
        with (
            tc.tile_pool(name="persist", bufs=1) as pp,
            tc.tile_pool(name="xp", bufs=1) as xp,
            tc.tile_pool(name="qp", bufs=1) as qp,
            tc.tile_pool(name="op", bufs=1) as op_,
            tc.tile_pool(name="wp", bufs=2) as wp,
            tc.tile_pool(name="wop", bufs=4) as wop,
            tc.tile_pool(name="ep", bufs=4) as ep,
            tc.tile_pool(name="tp", bufs=1) as tp,
            tc.tile_pool(name="mmps", bufs=2, space="PSUM") as mmps,
            tc.tile_pool(name="ops", bufs=2, space="PSUM") as ops_,
            tc.tile_pool(name="dps", bufs=1, space="PSUM") as dps,
            tc.tile_pool(name="vps_p", bufs=1, space="PSUM") as vps_p,
        ):
            # persistent SBUF tensors
            kT_p = {
                (kv, pi): pp.tile([128, 1024], bf16, tag=f"kT_{kv}_{pi}",
                                  name=f"kT_{kv}_{pi}")
                for kv in range(KVH) for pi in range(npair)
            }
            v_t = {
                lc: pp.tile([128, 4, KVH * HD], bf16, tag=f"v_{lc}", name=f"v_{lc}")
                for lc in range(nlc)
            }
            cs2 = pp.tile([128, seq_len], f32)
            sn2 = pp.tile([128, seq_len], f32)
            wvT = pp.tile([128, 32, KVH * HD], bf16)
            o128 = pp.tile([128, 128], bf16)

            nc.scalar.dma_start(cs2[0:64, :], cosT.ap())
            nc.scalar.dma_start(cs2[64:128, :], cosT.ap())
            nc.scalar.dma_start(sn2[0:64, :], sinT.ap())
            nc.scalar.dma_start(sn2[64:128, :], sinT.ap())
            # rotate-half form: out = t*cs2 + swap(t)*sn2 with sn2 = [-sin | sin]
            nc.vector.tensor_scalar_mul(sn2[0:64, :], sn2[0:64, :], -1.0)
            nc.scalar.dma_start(wvT[:], wvT_tl.ap())
            nc.scalar.dma_start(o128[:], ones128.ap())

            for pi in range(npair):
                cols = slice(pi * 1024, (pi + 1) * 1024)
                # ---- phase 1: x DMA + K/Q projections + RoPE ----
                x_p = xp.tile([128, 32, 1024], bf16, tag="x")
                for quar in range(4):
                    nc.sync.dma_start(
                        x_p[:, quar * 8:(quar + 1) * 8, :], x_tl.ap()[pi * 4 + quar]
                    )
                q_pr = qp.tile([128, QH, 1024], bf16, tag="q")
                o2 = op_.tile([128, QH, 1024], bf16, tag="o2")
                for mi in range(KVH + QH):      # k0,k1,q0..q7
                    wt = wp.tile([128, 32 * 128], bf16, tag="w")
                    nc.sync.dma_start(wt[:, :2048], wqk_tl.ap()[mi][:, :2048])
                    nc.sync.dma_start(wt[:, 2048:], wqk_tl.ap()[mi][:, 2048:])
                    ps2 = mmps.tile([128, 1024], f32, tag="big")
                    for dt in range(32):
                        for lci in range(2):
                            nc.tensor.matmul(
                                ps2[:, lci * 512:(lci + 1) * 512],
                                wt[:, dt * 128:(dt + 1) * 128],
                                x_p[:, dt, lci * 512:(lci + 1) * 512],
                                start=(dt == 0), stop=(dt == 31),
                            )
                    t1 = tp.tile([128, 1024], f32, tag="t1")
                    nc.vector.tensor_mul(t1[:], ps2[:], cs2[:, cols])
                    t2 = tp.tile([128, 1024], f32, tag="t2")
                    nc.vector.tensor_mul(t2[0:64, :], ps2[64:128, :], sn2[0:64, cols])
                    nc.vector.tensor_mul(t2[64:128, :], ps2[0:64, :], sn2[64:128, cols])
                    dst = kT_p[(mi, pi)][:] if mi < KVH else q_pr[:, mi - KVH, :]
                    nc.vector.tensor_tensor(dst, t1[:], t2[:], mybir.AluOpType.add)

                # ---- phase 1.5: V^T directly (stationary x, moving wv^T) ----
                for lci in range(2):
                    lc = 2 * pi + lci
                    for jj2 in range(2):
                        vps = vps_p.tile([128, 512], f32, tag="vps")
                        for t in range(2):
                            jt = jj2 * 2 + t
                            lcol = lci * 512 + jt * 128
                            for dt in range(32):
                                nc.tensor.matmul(
                                    vps[:, t * 256:(t + 1) * 256],
                                    x_p[:, dt, lcol:lcol + 128],
                                    wvT[:, dt, :],
                                    start=(dt == 0), stop=(dt == 31),
                                )
                        nc.vector.tensor_copy(
                            v_t[lc][:, jj2 * 2:(jj2 + 1) * 2, :],
                            vps[:].rearrange("p (a b) -> p a b", a=2),
                        )

                # ---- phase 2: causal attention, software-pipelined ----
                for lci in range(2):
                    lc = 2 * pi + lci
                    njt = 4 * (lc + 1)
                    ngrp = njt // 2
                    items = [(h, g) for h in range(QH) for g in range(ngrp)]

                    def emit_scores(h, g, lc=lc, lci=lci, njt=njt):
                        kv = h // (QH // KVH)
                        psS2 = mmps.tile([128, 1024], f32, tag="big")
                        for t in range(2):
                            jt = 2 * g + t
                            nc.tensor.matmul(
                                psS2[:, t * 512:(t + 1) * 512],
                                kT_p[(kv, jt // 8)][:, (jt % 8) * 128:(jt % 8 + 1) * 128],
                                q_pr[:, h, lci * 512:(lci + 1) * 512],
                                start=True, stop=True,
                            )
                        e = ep.tile([128, 1024], bf16, tag="e")
                        nc.scalar.activation(
                            e[:], psS2[:], mybir.ActivationFunctionType.Exp,
                            scale=SCALE,
                        )
                        for t in range(2):
                            jt = 2 * g + t
                            dg = jt - 4 * lc
                            if dg >= 0:
                                # causal: zero E where key j > query l
                                nc.gpsimd.affine_select(
                                    out=e[:, t * 512:(t + 1) * 512],
                                    in_=e[:, t * 512:(t + 1) * 512],
                                    compare_op=mybir.AluOpType.is_ge,
                                    fill=0.0,
                                    base=-128 * dg,
                                    pattern=[[1, 512]],
                                    channel_multiplier=-1,
                                )
                        return e

                    po_t = {}
                    pden_t = {}
                    ebuf = {}
                    for idx in range(len(items) + LOOKAHEAD):
                        if idx < len(items):
                            ebuf[idx] = emit_scores(*items[idx])
                        j = idx - LOOKAHEAD
                        if j < 0:
                            continue
                        h, g = items[j]
                        e = ebuf.pop(j)
                        if g == 0:
                            po_t[h] = ops_.tile([128, 512], f32, tag="po", name=f"po{h}")
                            pden_t[h] = dps.tile([128, 512], f32, tag="pden", name=f"pden{h}")
                        for t in range(2):
                            jt = 2 * g + t
                            nc.tensor.matmul(
                                po_t[h],
                                v_t[jt // 4][:, jt % 4, (h // (QH // KVH)) * 128:
                                             (h // (QH // KVH) + 1) * 128],
                                e[:, t * 512:(t + 1) * 512],
                                start=(jt == 0), stop=(jt == njt - 1),
                            )
                        for t in range(2):
                            jt = 2 * g + t
                            nc.tensor.matmul(
                                pden_t[h], o128[:], e[:, t * 512:(t + 1) * 512],
                                start=(jt == 0), stop=(jt == njt - 1),
                            )
                        if g == ngrp - 1:
                            rec = tp.tile([128, 512], f32, tag="rec", bufs=2)
                            nc.vector.reciprocal_approx_fast(out=rec[:], in_=pden_t[h])
                            nc.vector.tensor_mul(
                                o2[:, h, lci * 512:(lci + 1) * 512], po_t[h], rec[:]
                            )

                # ---- phase 3: partial output projection for the pair ----
                for nt in range(D // 128):
                    wo_t = wop.tile([128, QH * 128], bf16, tag="wo")
                    nc.scalar.dma_start(wo_t[:], wo_tl.ap()[nt])
                    pso2 = mmps.tile([128, 1024], f32, tag="big")
                    for h in range(QH):
                        for lci in range(2):
                            nc.tensor.matmul(
                                pso2[:, lci * 512:(lci + 1) * 512],
                                wo_t[:, h * 128:(h + 1) * 128],
                                o2[:, h, lci * 512:(lci + 1) * 512],
                                start=(h == 0), stop=(h == QH - 1),
                            )
                    ob = tp.tile([128, 1024], f32, tag="ob", bufs=2, name=f"ob{nt}")
                    nc.scalar.mul(ob[:], pso2[:], 1.0)
                    nc.sync.dma_start(
                        outT.ap()[nt * 128:(nt + 1) * 128, pi * 1024:(pi + 1) * 1024],
                        ob[:],
                    )
    nc.compile()
    return nc


_PERM = np.concatenate([np.arange(0, HD, 2), np.arange(1, HD, 2)])


def shard_inputs(x, wq, wk, wv, wo, cos, sin, mask, seq_len=L):
    """Build the 8 per-core input maps (host pre-tiling, bf16)."""
    nlc = seq_len // LC
    npair = nlc // 2
    cosT = np.ascontiguousarray(cos[:seq_len].T, dtype=np.float32)
    sinT = np.ascontiguousarray(sin[:seq_len].T, dtype=np.float32)
    ones128 = np.ones((128, 128), bfdt)

    x_tls = []
    for b in range(B):
        xT = np.asarray(x[b, :seq_len]).T.astype(np.float32)   # [D, seq]
        xv = xT.reshape(32, 128, nlc, 512)                     # [dt, p, lc, c]
        x_tl = np.zeros((npair * 4, 128, 8, 1024), bfdt)
        for pi in range(npair):
            blk = xv[:, :, 2 * pi:2 * pi + 2, :].reshape(4, 8, 128, 1024)
            x_tl[pi * 4:(pi + 1) * 4] = blk.transpose(0, 2, 1, 3)
        x_tls.append(x_tl)

    def permute_rows(w):
        nh = w.shape[0] // HD
        wp_ = w.reshape(nh, HD, -1)[:, _PERM, :]
        return wp_.reshape(w.shape)

    in_maps = []
    for c in range(NCORES):
        b, g = divmod(c, GROUPS)
        wq_g = permute_rows(np.asarray(wq)[QH * HD * g:QH * HD * (g + 1)])
        wk_g = permute_rows(np.asarray(wk)[KVH * HD * g:KVH * HD * (g + 1)])
        wv_g = np.asarray(wv)[KVH * HD * g:KVH * HD * (g + 1)]
        wo_g = np.asarray(wo)[:, QH * HD * g:QH * HD * (g + 1)]
        # [mi, p, dt*128+mc] = W[mi*128+mc, dt*128+p]
        W = np.concatenate([wk_g, wq_g], axis=0)               # [1280, 4096]
        wqk_tl = np.ascontiguousarray(
            W.reshape(KVH + QH, 128, 32, 128).transpose(0, 3, 2, 1)
            .reshape(KVH + QH, 128, 32 * 128), dtype=bfdt)
        # [p, dt, d] = wv_g[d, dt*128+p]
        wvT_tl = np.ascontiguousarray(
            wv_g.reshape(KVH * HD, 32, 128).transpose(2, 1, 0), dtype=bfdt)
        # [nt, p, h*128+n] = wo_g[nt*128+n, h*128+p]
        wo_tl = np.ascontiguousarray(
            wo_g.reshape(D // 128, 128, QH, 128).transpose(0, 3, 2, 1)
            .reshape(D // 128, 128, QH * 128), dtype=bfdt)
        in_maps.append({
            "x_tl": x_tls[b],
            "wqk_tl": wqk_tl,
            "wvT_tl": wvT_tl,
            "wo_tl": wo_tl,
            "cosT": cosT,
            "sinT": sinT,
            "ones128": ones128,
        })
    return in_maps


def gather_output(results, seq_len=L):
    out = np.zeros((B, seq_len, D), np.float32)
    for c in range(NCORES):
        b = c // GROUPS
        out[b] += results[c]["outT"].T
    return out


_nc_cache = {}


def _get_nc(seq_len=L):
    if seq_len not in _nc_cache:
        _nc_cache[seq_len] = build_nc(seq_len)
    return _nc_cache[seq_len]


def run_sharded(inputs, trace=False, tmpdir=None):
    nc = _get_nc()
    in_maps = shard_inputs(**inputs)
    res = bass_utils.run_bass_kernel_spmd(
        nc, in_maps, core_ids=list(range(NCORES)), trace=trace, tmpdir=tmpdir
    )
    return gather_output(res.results), res


def kernel(**inputs) -> np.ndarray:
    out, _ = run_sharded(inputs)
    return out


# revision 13
# speedup vs baseline: 1.5790x; 1.0053x over previous
"""GQA attention block (B=2, L=2048, D=4096, H=32, HKV=8, RoPE, causal) on 8
Trainium2 NeuronCores.

Sharding: core c -> batch b=c//4, head-group g=c%4 (8 Q heads + 2 KV heads per
core).  Each core computes x[b] @ wq_g/wk_g projections, V^T directly (by
swapping matmul operands: stationary x-tile, moving wv^T -> no transposes),
RoPE, causal attention for its heads, and a partial output projection against
its slice of wo; the host sums the 4 partials per batch element.

All DMA'd operands are bf16 (half the HBM traffic of fp32; TRN2 matmul rate is
1 row/cycle for bf16 and fp32r alike).  Scores are computed transposed
S^T[j,l] in pairs of key tiles per [128,1024] PSUM allocation so one exp
activation covers two tiles.  The softmax denominator stays off the PE: DVE
pair-sums accumulate eacc[j,l] per head, and a single all-ones stationary
matmul per (head, l-chunk) folds the 128 key partitions (partition-broadcast
denominator for free).  Causality: fully-masked key tiles are skipped;
diagonal tiles are zeroed post-exp with gpsimd affine_select.  The attention
loop is software-pipelined flat across (head, key-pair) items, and
independent PE work is interleaved into the Act-paced attention stretches:
V^T of the next l-pair during attention of the current one, the output
projection of the previous pair during the next pair's attention.  Output
partials are written bf16 (host upcasts and all-reduces).
"""

import numpy as np
import ml_dtypes

import concourse.mybir as mybir
import concourse.tile as tile
from concourse import bacc, bass_utils

B, L, D = 2, 2048, 4096
H, HKV, HD = 32, 8, 128
NCORES = 8
GROUPS = 4                # head groups (cores per batch element)
QH = H // GROUPS          # 8 q heads per core
KVH = HKV // GROUPS       # 2 kv heads per core
LC = 512                  # l-chunk
SCALE = 1.0 / float(np.sqrt(HD))
LOOKAHEAD = 2             # attention items (head, key-pair) in flight ahead of PV

f32 = mybir.dt.float32
f32r = mybir.dt.float32r
bf16 = mybir.dt.bfloat16
bfdt = ml_dtypes.bfloat16


def build_nc(seq_len=L):
    nlc = seq_len // LC            # 4
    npair = nlc // 2               # 2

    nc = bacc.Bacc(trn_type="TRN2")
    x_tl = nc.dram_tensor("x_tl", [npair * 2, 128, 32, 512], bf16, kind="ExternalInput")
    wqk_tl = nc.dram_tensor("wqk_tl", [KVH + QH, 128, 32 * 128], bf16, kind="ExternalInput")
    wvT_tl = nc.dram_tensor("wvT_tl", [128, 32, KVH * HD], bf16, kind="ExternalInput")
    wo_tl = nc.dram_tensor("wo_tl", [D // 128, 128, QH * 128], bf16, kind="ExternalInput")
    cosT = nc.dram_tensor("cosT", [64, seq_len], f32, kind="ExternalInput")
    sinT = nc.dram_tensor("sinT", [64, seq_len], f32, kind="ExternalInput")
    ones128 = nc.dram_tensor("ones128", [128, 128], f32, kind="ExternalInput")
    outT = nc.dram_tensor("outT", [D, seq_len], bf16, kind="ExternalOutput")

    with nc.allow_low_precision("bf16 attention kernel"), tile.TileContext(nc) as tc:
        with (
            tc.tile_pool(name="persist", bufs=1) as pp,
            tc.tile_pool(name="xp", bufs=2) as xp,
            tc.tile_pool(name="qp", bufs=1) as qp,
            tc.tile_pool(name="op", bufs=2) as op_,
            tc.tile_pool(name="wp", bufs=2) as wp,
            tc.tile_pool(name="wop", bufs=4) as wop,
            tc.tile_pool(name="ep", bufs=3) as ep,
            tc.tile_pool(name="tp", bufs=1) as tp,
            tc.tile_pool(name="mmps", bufs=2, space="PSUM") as mmps,
            tc.tile_pool(name="ops", bufs=2, space="PSUM") as ops_,
            tc.tile_pool(name="dps", bufs=1, space="PSUM") as dps,
            tc.tile_pool(name="vps_p", bufs=1, space="PSUM") as vps_p,
        ):
            # persistent SBUF tensors
            kT_p = {
                (kv, pi): pp.tile([128, 1024], bf16, tag=f"kT_{kv}_{pi}",
                                  name=f"kT_{kv}_{pi}")
                for kv in range(KVH) for pi in range(npair)
            }
            v_t = {
                lc: pp.tile([128, 4, KVH * HD], bf16, tag=f"v_{lc}", name=f"v_{lc}")
                for lc in range(nlc)
            }
            cs2 = pp.tile([128, seq_len], f32)
            sn2 = pp.tile([128, seq_len], f32)
            wvT = pp.tile([128, 32, KVH * HD], bf16)
            o128 = pp.tile([128, 128], f32r)

            # startup: first two weight tiles + first x chunks race on separate
            # queues; cos/sin etc. queue behind the weights on scalar.
            wts = {}
            for mi in range(2):
                wt = wp.tile([128, 32 * 128], bf16, tag="w", name=f"wt{mi}")
                nc.scalar.dma_start(wt[:, :2048], wqk_tl.ap()[mi][:, :2048])
                nc.scalar.dma_start(wt[:, 2048:], wqk_tl.ap()[mi][:, 2048:])
                wts[mi] = wt
            x_t = {}
            for lci in range(2):
                x_c = xp.tile([128, 32, 512], bf16, tag="x", name=f"x0_{lci}")
                for half in range(4):
                    nc.sync.dma_start(
                        x_c[:, half * 8:(half + 1) * 8, :],
                        x_tl.ap()[lci][:, half * 8:(half + 1) * 8, :],
                    )
                x_t[(0, lci)] = x_c
            nc.scalar.dma_start(cs2[0:64, :], cosT.ap())
            nc.scalar.dma_start(cs2[64:128, :], cosT.ap())
            nc.scalar.dma_start(sn2[0:64, :], sinT.ap())
            nc.scalar.dma_start(sn2[64:128, :], sinT.ap())
            # rotate-half form: out = t*cs2 + swap(t)*sn2 with sn2 = [-sin | sin]
            nc.vector.tensor_scalar_mul(sn2[0:64, :], sn2[0:64, :], -1.0)
            nc.scalar.dma_start(wvT[:], wvT_tl.ap())
            nc.scalar.dma_start(o128[:], ones128.ap().bitcast(f32r))

            # ---------------- emission generators ----------------

            def gen_projections(pi, q_pr):
                """K/Q projections + RoPE for pair pi; yields after each mi."""
                for mi in range(KVH + QH):      # k0,k1,q0..q7
                    if (pi, mi) == (0, 0) or (pi, mi) == (0, 1):
                        wt = wts[mi]
                    else:
                        wt = wp.tile([128, 32 * 128], bf16, tag="w",
                                     name=f"wt{pi}_{mi}")
                        nc.sync.dma_start(wt[:, :2048], wqk_tl.ap()[mi][:, :2048])
                        nc.sync.dma_start(wt[:, 2048:], wqk_tl.ap()[mi][:, 2048:])
                    ps2 = mmps.tile([128, 1024], f32, tag="big", name=f"ps{pi}_{mi}")
                    for dt in range(32):
                        for lci in range(2):
                            nc.tensor.matmul(
                                ps2[:, lci * 512:(lci + 1) * 512],
                                wt[:, dt * 128:(dt + 1) * 128],
                                x_t[(pi, lci)][:, dt, :],
                                start=(dt == 0), stop=(dt == 31),
                            )
                    cols = slice(pi * 1024, (pi + 1) * 1024)
                    t1 = tp.tile([128, 1024], f32, tag="t1", name=f"t1_{pi}_{mi}")
                    nc.vector.tensor_mul(t1[:], ps2[:], cs2[:, cols])
                    dst = kT_p[(mi, pi)][:] if mi < KVH else q_pr[:, mi - KVH, :]
                    nc.vector.tensor_mul(dst[0:64, :], ps2[64:128, :], sn2[0:64, cols])
                    nc.vector.tensor_mul(dst[64:128, :], ps2[0:64, :], sn2[64:128, cols])
                    nc.vector.tensor_tensor(dst, dst, t1[:], mybir.AluOpType.add)
                    yield

            def gen_vT(pi):
                """V^T for both l-chunks of pair pi; yields after each j-subtile."""
                for lci in range(2):
                    lc = 2 * pi + lci
                    for jj2 in range(2):
                        vps = vps_p.tile([128, 512], f32, tag="vps",
                                         name=f"vps{lc}_{jj2}")
                        for t in range(2):
                            jt = jj2 * 2 + t
                            for dt in range(32):
                                nc.tensor.matmul(
                                    vps[:, t * 256:(t + 1) * 256],
                                    x_t[(pi, lci)][:, dt, jt * 128:(jt + 1) * 128],
                                    wvT[:, dt, :],
                                    start=(dt == 0), stop=(dt == 31),
                                )
                            yield
                        nc.vector.tensor_copy(
                            v_t[lc][:, jj2 * 2:(jj2 + 1) * 2, :],
                            vps[:].rearrange("p (a b) -> p a b", a=2),
                        )

            def gen_attention(pi, q_pr, o2):
                """Causal attention for both l-chunks of pair pi, software-
                pipelined flat across (head, key-pair) items; yields per item."""
                for lci in range(2):
                    lc = 2 * pi + lci
                    njt = 4 * (lc + 1)
                    ngrp = njt // 2
                    items = [(h, g) for h in range(QH) for g in range(ngrp)]

                    def emit_scores(h, g, lc=lc, lci=lci):
                        kv = h // (QH // KVH)
                        psS2 = mmps.tile([128, 1024], f32, tag="big",
                                         name=f"psS{lc}_{h}_{g}")
                        for t in range(2):
                            jt = 2 * g + t
                            nc.tensor.matmul(
                                psS2[:, t * 512:(t + 1) * 512],
                                kT_p[(kv, jt // 8)][:, (jt % 8) * 128:(jt % 8 + 1) * 128],
                                q_pr[:, h, lci * 512:(lci + 1) * 512],
                                start=True, stop=True,
                            )
                        e = ep.tile([128, 1024], bf16, tag="e", name=f"e{lc}_{h}_{g}")
                        nc.scalar.activation(
                            e[:], psS2[:], mybir.ActivationFunctionType.Exp,
                            scale=SCALE,
                        )
                        for t in range(2):
                            jt = 2 * g + t
                            dg = jt - 4 * lc
                            if dg >= 0:
                                # causal: zero E where key j > query l
                                nc.gpsimd.affine_select(
                                    out=e[:, t * 512:(t + 1) * 512],
                                    in_=e[:, t * 512:(t + 1) * 512],
                                    compare_op=mybir.AluOpType.is_ge,
                                    fill=0.0,
                                    base=-128 * dg,
                                    pattern=[[1, 512]],
                                    channel_multiplier=-1,
                                )
                        return e

                    po_t, eacc_t, ebuf = {}, {}, {}
                    for idx in range(len(items) + LOOKAHEAD):
                        if idx < len(items):
                            ebuf[idx] = emit_scores(*items[idx])
                        j = idx - LOOKAHEAD
                        if j < 0:
                            yield
                            continue
                        h, g = items[j]
                        e = ebuf.pop(j)
                        if g == 0:
                            po_t[h] = ops_.tile([128, 512], f32, tag="po",
                                                name=f"po{lc}_{h}")
                            eacc_t[h] = tp.tile([128, 512], f32r, tag="eacc",
                                                bufs=2, name=f"eacc{lc}_{h}")
                        for t in range(2):
                            jt = 2 * g + t
                            nc.tensor.matmul(
                                po_t[h],
                                v_t[jt // 4][:, jt % 4, (h // (QH // KVH)) * 128:
                                             (h // (QH // KVH) + 1) * 128],
                                e[:, t * 512:(t + 1) * 512],
                                start=(jt == 0), stop=(jt == njt - 1),
                            )
                        # denominator accumulation on DVE (keys stay on partitions)
                        if g == 0:
                            nc.vector.tensor_tensor(
                                eacc_t[h][:], e[:, 0:512], e[:, 512:1024],
                                mybir.AluOpType.add,
                            )
                        else:
                            pg = tp.tile([128, 512], bf16, tag="pg", bufs=1,
                                         name=f"pg{lc}_{h}_{g}")
                            nc.vector.tensor_tensor(
                                pg[:], e[:, 0:512], e[:, 512:1024],
                                mybir.AluOpType.add,
                            )
                            nc.vector.tensor_tensor(
                                eacc_t[h][:], eacc_t[h][:], pg[:],
                                mybir.AluOpType.add,
                            )
                        if g == ngrp - 1:
                            # fold 128 key partitions with one ones-matmul
                            pden = dps.tile([128, 512], f32, tag="pden",
                                            name=f"pden{lc}_{h}")
                            nc.tensor.matmul(
                                pden[:], o128[:], eacc_t[h][:],
                                start=True, stop=True,
                            )
                            rec = tp.tile([128, 512], f32, tag="rec", bufs=1,
                                          name=f"rec{lc}_{h}")
                            nc.vector.reciprocal_approx_fast(out=rec[:], in_=pden[:])
                            nc.vector.tensor_mul(
                                o2[:, h, lci * 512:(lci + 1) * 512], po_t[h], rec[:]
                            )
                        yield

            def gen_outproj(pi, o2):
                """Partial output projection for pair pi; yields after each nt."""
                for nt in range(D // 128):
                    wo_t = wop.tile([128, QH * 128], bf16, tag="wo",
                                    name=f"wo{pi}_{nt}")
                    nc.scalar.dma_start(wo_t[:], wo_tl.ap()[nt])
                    pso2 = mmps.tile([128, 1024], f32, tag="big",
                                     name=f"pso{pi}_{nt}")
                    for h in range(QH):
                        for lci in range(2):
                            nc.tensor.matmul(
                                pso2[:, lci * 512:(lci + 1) * 512],
                                wo_t[:, h * 128:(h + 1) * 128],
                                o2[:, h, lci * 512:(lci + 1) * 512],
                                start=(h == 0), stop=(h == QH - 1),
                            )
                    ob = tp.tile([128, 1024], bf16, tag="ob", bufs=2,
                                 name=f"ob{pi}_{nt}")
                    nc.scalar.mul(ob[:], pso2[:], 1.0)
                    nc.sync.dma_start(
                        outT.ap()[nt * 128:(nt + 1) * 128, pi * 1024:(pi + 1) * 1024],
                        ob[:],
                    )
                    yield

            def drain(gen):
                for _ in gen:
                    pass

            def interleave(main_gen, fill_gen, ratio):
                """Emit ratio items of main_gen per item of fill_gen; main first,
                then drain both."""
                n = 0
                for _ in main_gen:
                    n += 1
                    if n % ratio == 0:
                        next(fill_gen, None)
                drain(fill_gen)

            # ---------------- schedule ----------------
            q_pr0 = qp.tile([128, QH, 1024], bf16, tag="q", name="q_pr0")
            o2_0 = op_.tile([128, QH, 1024], bf16, tag="o2", name="o2_0")
            drain(gen_projections(0, q_pr0))
            drain(gen_vT(0))

            # pair-1 x DMA early so interleaved V^T(1) has data
            for lci in range(2):
                x_c = xp.tile([128, 32, 512], bf16, tag="x", name=f"x1_{lci}")
                for half in range(2):
                    nc.sync.dma_start(
                        x_c[:, half * 16:(half + 1) * 16, :],
                        x_tl.ap()[2 + lci][:, half * 16:(half + 1) * 16, :],
                    )
                x_t[(1, lci)] = x_c

            # attention(pair0) with V^T(pair1) filling Act-paced gaps
            interleave(gen_attention(0, q_pr0, o2_0), gen_vT(1), 6)

            q_pr1 = qp.tile([128, QH, 1024], bf16, tag="q", name="q_pr1")
            o2_1 = op_.tile([128, QH, 1024], bf16, tag="o2", name="o2_1")
            drain(gen_projections(1, q_pr1))

            # attention(pair1) with outproj(pair0) filling gaps
            interleave(gen_attention(1, q_pr1, o2_1), gen_outproj(0, o2_0), 4)

            drain(gen_outproj(1, o2_1))
    nc.compile()
    return nc


_PERM = np.concatenate([np.arange(0, HD, 2), np.arange(1, HD, 2)])


def shard_inputs(x, wq, wk, wv, wo, cos, sin, mask, seq_len=L):
    """Build the 8 per-core input maps (host pre-tiling, bf16)."""
    nlc = seq_len // LC
    cosT = np.ascontiguousarray(cos[:seq_len].T, dtype=np.float32)
    sinT = np.ascontiguousarray(sin[:seq_len].T, dtype=np.float32)
    ones128 = np.ones((128, 128), np.float32)

    x_tls = []
    for b in range(B):
        xT = np.asarray(x[b, :seq_len]).T.astype(np.float32)   # [D, seq]
        # [lc, p, dt, c] = x[b, lc*512+c, dt*128+p]
        xv = xT.reshape(32, 128, nlc, 512).transpose(2, 1, 0, 3)
        x_tls.append(np.ascontiguousarray(xv, dtype=bfdt))

    def permute_rows(w):
        nh = w.shape[0] // HD
        wp_ = w.reshape(nh, HD, -1)[:, _PERM, :]
        return wp_.reshape(w.shape)

    in_maps = []
    for c in range(NCORES):
        b, g = divmod(c, GROUPS)
        wq_g = permute_rows(np.asarray(wq)[QH * HD * g:QH * HD * (g + 1)])
        wk_g = permute_rows(np.asarray(wk)[KVH * HD * g:KVH * HD * (g + 1)])
        wv_g = np.asarray(wv)[KVH * HD * g:KVH * HD * (g + 1)]
        wo_g = np.asarray(wo)[:, QH * HD * g:QH * HD * (g + 1)]
        # [mi, p, dt*128+mc] = W[mi*128+mc, dt*128+p]
        W = np.concatenate([wk_g, wq_g], axis=0)               # [1280, 4096]
        wqk_tl = np.ascontiguousarray(
            W.reshape(KVH + QH, 128, 32, 128).transpose(0, 3, 2, 1)
            .reshape(KVH + QH, 128, 32 * 128), dtype=bfdt)
        # [p, dt, d] = wv_g[d, dt*128+p]
        wvT_tl = np.ascontiguousarray(
            wv_g.reshape(KVH * HD, 32, 128).transpose(2, 1, 0), dtype=bfdt)
        # [nt, p, h*128+n] = wo_g[nt*128+n, h*128+p]
        wo_tl = np.ascontiguousarray(
            wo_g.reshape(D // 128, 128, QH, 128).transpose(0, 3, 2, 1)
            .reshape(D // 128, 128, QH * 128), dtype=bfdt)
        in_maps.append({
            "x_tl": x_tls[b],
            "wqk_tl": wqk_tl,
            "wvT_tl": wvT_tl,
            "wo_tl": wo_tl,
            "cosT": cosT,
            "sinT": sinT,
            "ones128": ones128,
        })
    return in_maps


def gather_output(results, seq_len=L):
    out = np.zeros((B, seq_len, D), np.float32)
    for c in range(NCORES):
        b = c // GROUPS
        out[b] += np.asarray(results[c]["outT"], dtype=np.float32).T
    return out


_nc_cache = {}


def _get_nc(seq_len=L):
    if seq_len not in _nc_cache:
        _nc_cache[seq_len] = build_nc(seq_len)
    return _nc_cache[seq_len]


def run_sharded(inputs, trace=False, tmpdir=None):
    nc = _get_nc()
    in_maps = shard_inputs(**inputs)
    res = bass_utils.run_bass_kernel_spmd(
        nc, in_maps, core_ids=list(range(NCORES)), trace=trace, tmpdir=tmpdir
    )
    return gather_output(res.results), res


def kernel(**inputs) -> np.ndarray:
    out, _ = run_sharded(inputs)
    return out


# revision 14
# speedup vs baseline: 1.6152x; 1.0229x over previous
"""GQA attention block (B=2, L=2048, D=4096, H=32, HKV=8, RoPE, causal) on 8
Trainium2 NeuronCores.

Sharding: core c -> batch b=c//4, head-group g=c%4 (8 Q heads + 2 KV heads per
core).  Each core computes x[b] @ wq_g/wk_g projections, V^T directly (by
swapping matmul operands: stationary x-tile, moving wv^T -> no transposes),
RoPE, causal attention for its heads, and a partial output projection against
its slice of wo; the host sums the 4 partials per batch element.

All DMA'd operands are bf16 (half the HBM traffic of fp32; TRN2 matmul rate is
1 row/cycle for bf16 and fp32r alike).  Scores are computed transposed
S^T[j,l] in pairs of key tiles per [128,1024] PSUM allocation so one exp
activation covers two tiles.  The softmax denominator stays off the PE: DVE
pair-sums accumulate eacc[j,l] per head, and a single all-ones stationary
matmul per (head, l-chunk) folds the 128 key partitions (partition-broadcast
denominator for free).  Causality: fully-masked key tiles are skipped;
diagonal tiles are zeroed post-exp with gpsimd affine_select.  The attention
loop is software-pipelined flat across (head, key-pair) items, and
independent PE work is interleaved into the Act-paced attention stretches:
V^T of the next l-pair during attention of the current one, the output
projection of the previous pair during the next pair's attention.  Output
partials are written bf16 (host upcasts and all-reduces).
"""

import numpy as np
import ml_dtypes

import concourse.mybir as mybir
import concourse.tile as tile
from concourse import bacc, bass_utils

B, L, D = 2, 2048, 4096
H, HKV, HD = 32, 8, 128
NCORES = 8
GROUPS = 4                # head groups (cores per batch element)
QH = H // GROUPS          # 8 q heads per core
KVH = HKV // GROUPS       # 2 kv heads per core
LC = 512                  # l-chunk
SCALE = 1.0 / float(np.sqrt(HD))
LOOKAHEAD = 2             # attention items (head, key-pair) in flight ahead of PV

f32 = mybir.dt.float32
f32r = mybir.dt.float32r
bf16 = mybir.dt.bfloat16
bfdt = ml_dtypes.bfloat16


def build_nc(seq_len=L):
    nlc = seq_len // LC            # 4
    npair = nlc // 2               # 2

    nc = bacc.Bacc(trn_type="TRN2")
    x_tl = nc.dram_tensor("x_tl", [npair * 2, 128, 32, 512], bf16, kind="ExternalInput")
    wqk_tl = nc.dram_tensor("wqk_tl", [KVH + QH, 128, 32 * 128], bf16, kind="ExternalInput")
    wvT_tl = nc.dram_tensor("wvT_tl", [128, 32, KVH * HD], bf16, kind="ExternalInput")
    wo_tl = nc.dram_tensor("wo_tl", [D // 128, 128, QH * 128], bf16, kind="ExternalInput")
    cosT = nc.dram_tensor("cosT", [64, seq_len], f32, kind="ExternalInput")
    sinT = nc.dram_tensor("sinT", [64, seq_len], f32, kind="ExternalInput")
    ones128 = nc.dram_tensor("ones128", [128, 128], f32, kind="ExternalInput")
    outT = nc.dram_tensor("outT", [D, seq_len], bf16, kind="ExternalOutput")

    with nc.allow_low_precision("bf16 attention kernel"), tile.TileContext(nc) as tc:
        with (
            tc.tile_pool(name="persist", bufs=1) as pp,
            tc.tile_pool(name="xp", bufs=2) as xp,
            tc.tile_pool(name="qp", bufs=1) as qp,
            tc.tile_pool(name="op", bufs=2) as op_,
            tc.tile_pool(name="wp", bufs=2) as wp,
            tc.tile_pool(name="wop", bufs=4) as wop,
            tc.tile_pool(name="ep", bufs=3) as ep,
            tc.tile_pool(name="tp", bufs=1) as tp,
            tc.tile_pool(name="mmps", bufs=2, space="PSUM") as mmps,
            tc.tile_pool(name="ops", bufs=2, space="PSUM") as ops_,
            tc.tile_pool(name="dps", bufs=1, space="PSUM") as dps,
            tc.tile_pool(name="vps_p", bufs=1, space="PSUM") as vps_p,
        ):
            # persistent SBUF tensors
            kT_p = {
                (kv, pi): pp.tile([128, 1024], bf16, tag=f"kT_{kv}_{pi}",
                                  name=f"kT_{kv}_{pi}")
                for kv in range(KVH) for pi in range(npair)
            }
            v_t = {
                lc: pp.tile([128, 4, KVH * HD], bf16, tag=f"v_{lc}", name=f"v_{lc}")
                for lc in range(nlc)
            }
            cs2 = pp.tile([128, seq_len], f32)
            sn2 = pp.tile([128, seq_len], f32)
            wvT = pp.tile([128, 32, KVH * HD], bf16)
            o128 = pp.tile([128, 128], f32r)

            # startup: first two weight tiles + first x chunks race on separate
            # queues; cos/sin etc. queue behind the weights on scalar.
            wts = {}
            for mi in range(2):
                wt = wp.tile([128, 32 * 128], bf16, tag="w", name=f"wt{mi}")
                nc.scalar.dma_start(wt[:, :2048], wqk_tl.ap()[mi][:, :2048])
                nc.scalar.dma_start(wt[:, 2048:], wqk_tl.ap()[mi][:, 2048:])
                wts[mi] = wt
            x_t = {}
            for lci in range(2):
                x_t[(0, lci)] = xp.tile([128, 32, 512], bf16, tag="x",
                                        name=f"x0_{lci}")
            for half in range(4):
                for lci in range(2):
                    nc.sync.dma_start(
                        x_t[(0, lci)][:, half * 8:(half + 1) * 8, :],
                        x_tl.ap()[lci][:, half * 8:(half + 1) * 8, :],
                    )
            nc.scalar.dma_start(cs2[0:64, :], cosT.ap())
            nc.scalar.dma_start(cs2[64:128, :], cosT.ap())
            nc.scalar.dma_start(sn2[0:64, :], sinT.ap())
            nc.scalar.dma_start(sn2[64:128, :], sinT.ap())
            # rotate-half form: out = t*cs2 + swap(t)*sn2 with sn2 = [-sin | sin]
            nc.vector.tensor_scalar_mul(sn2[0:64, :], sn2[0:64, :], -1.0)
            nc.scalar.dma_start(wvT[:], wvT_tl.ap())
            nc.scalar.dma_start(o128[:], ones128.ap().bitcast(f32r))

            # ---------------- emission generators ----------------

            def gen_projections(pi, q_pr):
                """K/Q projections + RoPE for pair pi; yields after each mi."""
                for mi in range(KVH + QH):      # k0,k1,q0..q7
                    if (pi, mi) == (0, 0) or (pi, mi) == (0, 1):
                        wt = wts[mi]
                    else:
                        wt = wp.tile([128, 32 * 128], bf16, tag="w",
                                     name=f"wt{pi}_{mi}")
                        nc.sync.dma_start(wt[:, :2048], wqk_tl.ap()[mi][:, :2048])
                        nc.sync.dma_start(wt[:, 2048:], wqk_tl.ap()[mi][:, 2048:])
                    ps2 = mmps.tile([128, 1024], f32, tag="big", name=f"ps{pi}_{mi}")
                    for dt in range(32):
                        for lci in range(2):
                            nc.tensor.matmul(
                                ps2[:, lci * 512:(lci + 1) * 512],
                                wt[:, dt * 128:(dt + 1) * 128],
                                x_t[(pi, lci)][:, dt, :],
                                start=(dt == 0), stop=(dt == 31),
                            )
                    cols = slice(pi * 1024, (pi + 1) * 1024)
                    t1 = tp.tile([128, 1024], f32, tag="t1", name=f"t1_{pi}_{mi}")
                    nc.vector.tensor_mul(t1[:], ps2[:], cs2[:, cols])
                    dst = kT_p[(mi, pi)][:] if mi < KVH else q_pr[:, mi - KVH, :]
                    nc.vector.tensor_mul(dst[0:64, :], ps2[64:128, :], sn2[0:64, cols])
                    nc.vector.tensor_mul(dst[64:128, :], ps2[0:64, :], sn2[64:128, cols])
                    nc.vector.tensor_tensor(dst, dst, t1[:], mybir.AluOpType.add)
                    yield

            def gen_vT(pi):
                """V^T for both l-chunks of pair pi; yields after each j-subtile."""
                for lci in range(2):
                    lc = 2 * pi + lci
                    for jj2 in range(2):
                        vps = vps_p.tile([128, 512], f32, tag="vps",
                                         name=f"vps{lc}_{jj2}")
                        for t in range(2):
                            jt = jj2 * 2 + t
                            for dt in range(32):
                                nc.tensor.matmul(
                                    vps[:, t * 256:(t + 1) * 256],
                                    x_t[(pi, lci)][:, dt, jt * 128:(jt + 1) * 128],
                                    wvT[:, dt, :],
                                    start=(dt == 0), stop=(dt == 31),
                                )
                            yield
                        nc.vector.tensor_copy(
                            v_t[lc][:, jj2 * 2:(jj2 + 1) * 2, :],
                            vps[:].rearrange("p (a b) -> p a b", a=2),
                        )

            def gen_attention(pi, q_pr, o2):
                """Causal attention for both l-chunks of pair pi, software-
                pipelined flat across (head, key-pair) items; yields per item."""
                for lci in range(2):
                    lc = 2 * pi + lci
                    njt = 4 * (lc + 1)
                    ngrp = njt // 2
                    items = [(h, g) for h in range(QH) for g in range(ngrp)]

                    def emit_scores(h, g, lc=lc, lci=lci):
                        kv = h // (QH // KVH)
                        psS2 = mmps.tile([128, 1024], f32, tag="big",
                                         name=f"psS{lc}_{h}_{g}")
                        for t in range(2):
                            jt = 2 * g + t
                            nc.tensor.matmul(
                                psS2[:, t * 512:(t + 1) * 512],
                                kT_p[(kv, jt // 8)][:, (jt % 8) * 128:(jt % 8 + 1) * 128],
                                q_pr[:, h, lci * 512:(lci + 1) * 512],
                                start=True, stop=True,
                            )
                        e = ep.tile([128, 1024], bf16, tag="e", name=f"e{lc}_{h}_{g}")
                        nc.scalar.activation(
                            e[:], psS2[:], mybir.ActivationFunctionType.Exp,
                            scale=SCALE,
                        )
                        for t in range(2):
                            jt = 2 * g + t
                            dg = jt - 4 * lc
                            if dg >= 0:
                                # causal: zero E where key j > query l
                                nc.gpsimd.affine_select(
                                    out=e[:, t * 512:(t + 1) * 512],
                                    in_=e[:, t * 512:(t + 1) * 512],
                                    compare_op=mybir.AluOpType.is_ge,
                                    fill=0.0,
                                    base=-128 * dg,
                                    pattern=[[1, 512]],
                                    channel_multiplier=-1,
                                )
                        return e

                    po_t, eacc_t, ebuf = {}, {}, {}
                    for idx in range(len(items) + LOOKAHEAD):
                        if idx < len(items):
                            ebuf[idx] = emit_scores(*items[idx])
                        j = idx - LOOKAHEAD
                        if j < 0:
                            yield
                            continue
                        h, g = items[j]
                        e = ebuf.pop(j)
                        if g == 0:
                            po_t[h] = ops_.tile([128, 512], f32, tag="po",
                                                name=f"po{lc}_{h}")
                            eacc_t[h] = tp.tile([128, 512], f32r, tag="eacc",
                                                bufs=2, name=f"eacc{lc}_{h}")
                        for t in range(2):
                            jt = 2 * g + t
                            nc.tensor.matmul(
                                po_t[h],
                                v_t[jt // 4][:, jt % 4, (h // (QH // KVH)) * 128:
                                             (h // (QH // KVH) + 1) * 128],
                                e[:, t * 512:(t + 1) * 512],
                                start=(jt == 0), stop=(jt == njt - 1),
                            )
                        # denominator accumulation on DVE (keys stay on partitions)
                        if g == 0:
                            nc.vector.tensor_tensor(
                                eacc_t[h][:], e[:, 0:512], e[:, 512:1024],
                                mybir.AluOpType.add,
                            )
                        else:
                            pg = tp.tile([128, 512], bf16, tag="pg", bufs=1,
                                         name=f"pg{lc}_{h}_{g}")
                            nc.vector.tensor_tensor(
                                pg[:], e[:, 0:512], e[:, 512:1024],
                                mybir.AluOpType.add,
                            )
                            nc.vector.tensor_tensor(
                                eacc_t[h][:], eacc_t[h][:], pg[:],
                                mybir.AluOpType.add,
                            )
                        if g == ngrp - 1:
                            # fold 128 key partitions with one ones-matmul
                            pden = dps.tile([128, 512], f32, tag="pden",
                                            name=f"pden{lc}_{h}")
                            nc.tensor.matmul(
                                pden[:], o128[:], eacc_t[h][:],
                                start=True, stop=True,
                            )
                            rec = tp.tile([128, 512], f32, tag="rec", bufs=1,
                                          name=f"rec{lc}_{h}")
                            nc.vector.reciprocal_approx_fast(out=rec[:], in_=pden[:])
                            nc.vector.tensor_mul(
                                o2[:, h, lci * 512:(lci + 1) * 512], po_t[h], rec[:]
                            )
                        yield

            def gen_outproj(pi, o2, interleaved=False):
                """Partial output projection for pair pi; yields after each nt.
                The interleaved variant must not block the Act queue (exp lives
                there): weight DMAs go on sync, the PSUM drain on DVE."""
                for nt in range(D // 128):
                    wo_t = wop.tile([128, QH * 128], bf16, tag="wo",
                                    name=f"wo{pi}_{nt}")
                    (nc.sync if interleaved else nc.scalar).dma_start(
                        wo_t[:], wo_tl.ap()[nt])
                    pso2 = mmps.tile([128, 1024], f32, tag="big",
                                     name=f"pso{pi}_{nt}")
                    for h in range(QH):
                        for lci in range(2):
                            nc.tensor.matmul(
                                pso2[:, lci * 512:(lci + 1) * 512],
                                wo_t[:, h * 128:(h + 1) * 128],
                                o2[:, h, lci * 512:(lci + 1) * 512],
                                start=(h == 0), stop=(h == QH - 1),
                            )
                    ob = tp.tile([128, 1024], bf16, tag="ob", bufs=2,
                                 name=f"ob{pi}_{nt}")
                    if interleaved:
                        nc.vector.tensor_copy(ob[:], pso2[:])
                    else:
                        nc.scalar.mul(ob[:], pso2[:], 1.0)
                    nc.sync.dma_start(
                        outT.ap()[nt * 128:(nt + 1) * 128, pi * 1024:(pi + 1) * 1024],
                        ob[:],
                    )
                    yield

            def drain(gen):
                for _ in gen:
                    pass

            def interleave(main_gen, fill_gen, ratio):
                """Emit ratio items of main_gen per item of fill_gen; main first,
                then drain both."""
                n = 0
                for _ in main_gen:
                    n += 1
                    if n % ratio == 0:
                        next(fill_gen, None)
                drain(fill_gen)

            # ---------------- schedule ----------------
            q_pr0 = qp.tile([128, QH, 1024], bf16, tag="q", name="q_pr0")
            o2_0 = op_.tile([128, QH, 1024], bf16, tag="o2", name="o2_0")
            drain(gen_projections(0, q_pr0))
            drain(gen_vT(0))

            # pair-1 x DMA early so interleaved V^T(1) has data
            for lci in range(2):
                x_c = xp.tile([128, 32, 512], bf16, tag="x", name=f"x1_{lci}")
                for half in range(2):
                    nc.sync.dma_start(
                        x_c[:, half * 16:(half + 1) * 16, :],
                        x_tl.ap()[2 + lci][:, half * 16:(half + 1) * 16, :],
                    )
                x_t[(1, lci)] = x_c

            # attention(pair0) with V^T(pair1) filling Act-paced gaps
            interleave(gen_attention(0, q_pr0, o2_0), gen_vT(1), 6)

            q_pr1 = qp.tile([128, QH, 1024], bf16, tag="q", name="q_pr1")
            o2_1 = op_.tile([128, QH, 1024], bf16, tag="o2", name="o2_1")
            drain(gen_projections(1, q_pr1))

            # attention(pair1) with outproj(pair0) filling gaps
            interleave(gen_attention(1, q_pr1, o2_1), gen_outproj(0, o2_0, interleaved=True), 4)

            drain(gen_outproj(1, o2_1))
    nc.compile()
    return nc


_PERM = np.concatenate([np.arange(0, HD, 2), np.arange(1, HD, 2)])


def shard_inputs(x, wq, wk, wv, wo, cos, sin, mask, seq_len=L):
    """Build the 8 per-core input maps (host pre-tiling, bf16)."""
    nlc = seq_len // LC
    cosT = np.ascontiguousarray(cos[:seq_len].T, dtype=np.float32)
    sinT = np.ascontiguousarray(sin[:seq_len].T, dtype=np.float32)
    ones128 = np.ones((128, 128), np.float32)

    x_tls = []
    for b in range(B):
        xT = np.asarray(x[b, :seq_len]).T.astype(np.float32)   # [D, seq]
        # [lc, p, dt, c] = x[b, lc*512+c, dt*128+p]
        xv = xT.reshape(32, 128, nlc, 512).transpose(2, 1, 0, 3)
        x_tls.append(np.ascontiguousarray(xv, dtype=bfdt))

    def permute_rows(w):
        nh = w.shape[0] // HD
        wp_ = w.reshape(nh, HD, -1)[:, _PERM, :]
        return wp_.reshape(w.shape)

    in_maps = []
    for c in range(NCORES):
        b, g = divmod(c, GROUPS)
        wq_g = permute_rows(np.asarray(wq)[QH * HD * g:QH * HD * (g + 1)])
        wk_g = permute_rows(np.asarray(wk)[KVH * HD * g:KVH * HD * (g + 1)])
        wv_g = np.asarray(wv)[KVH * HD * g:KVH * HD * (g + 1)]
        wo_g = np.asarray(wo)[:, QH * HD * g:QH * HD * (g + 1)]
        # [mi, p, dt*128+mc] = W[mi*128+mc, dt*128+p]
        W = np.concatenate([wk_g, wq_g], axis=0)               # [1280, 4096]
        wqk_tl = np.ascontiguousarray(
            W.reshape(KVH + QH, 128, 32, 128).transpose(0, 3, 2, 1)
            .reshape(KVH + QH, 128, 32 * 128), dtype=bfdt)
        # [p, dt, d] = wv_g[d, dt*128+p]
        wvT_tl = np.ascontiguousarray(
            wv_g.reshape(KVH * HD, 32, 128).transpose(2, 1, 0), dtype=bfdt)
        # [nt, p, h*128+n] = wo_g[nt*128+n, h*128+p]
        wo_tl = np.ascontiguousarray(
            wo_g.reshape(D // 128, 128, QH, 128).transpose(0, 3, 2, 1)
            .reshape(D // 128, 128, QH * 128), dtype=bfdt)
        in_maps.append({
            "x_tl": x_tls[b],
            "wqk_tl": wqk_tl,
            "wvT_tl": wvT_tl,
            "wo_tl": wo_tl,
            "cosT": cosT,
            "sinT": sinT,
            "ones128": ones128,
        })
    return in_maps


def gather_output(results, seq_len=L):
    out = np.zeros((B, seq_len, D), np.float32)
    for c in range(NCORES):
        b = c // GROUPS
        out[b] += np.asarray(results[c]["outT"], dtype=np.float32).T
    return out


_nc_cache = {}


def _get_nc(seq_len=L):
    if seq_len not in _nc_cache:
        _nc_cache[seq_len] = build_nc(seq_len)
    return _nc_cache[seq_len]


def run_sharded(inputs, trace=False, tmpdir=None):
    nc = _get_nc()
    in_maps = shard_inputs(**inputs)
    res = bass_utils.run_bass_kernel_spmd(
        nc, in_maps, core_ids=list(range(NCORES)), trace=trace, tmpdir=tmpdir
    )
    return gather_output(res.results), res


def kernel(**inputs) -> np.ndarray:
    out, _ = run_sharded(inputs)
    return out
